# revision 14
# baseline (speedup 1.0000x reference)
"""EpisodicMemory kernel for Trainium2 (8 NeuronCores, data-parallel over BSB).

Each core processes 2 of the 16 batch-block rows:
  read:  masked top-32 attention over M=1024 slots + grouped linears
  novelty scoring, top-64 candidate selection
  write: softmax slot scoring + EMA blend of em_K/em_V/em_S/em_age
"""
import sys
sys.path.insert(0, '/opt/trn_rl_repo')

import contextlib
import numpy as np
import concourse.bass as bass
import concourse.tile as tile
from concourse import mybir
from concourse.bass_utils import run_bass_kernel_spmd
from concourse.masks import make_identity
import tile_patch  # noqa: F401  — splits >1-wait instructions for this walrus

F32 = mybir.dt.float32
F32R = mybir.dt.float32r
I32 = mybir.dt.int32
Alu = mybir.AluOpType
Act = mybir.ActivationFunctionType
AxX = mybir.AxisListType.X

NC_Q = 2048
D = 128
M = 1024
C_CAND = 64
S_MAX = 4.0
BUDGET = 64.0
NEG_REPL = -1e30
EXACT_THR = True   # True: exact 4-round topk32; False: 32-chunk prefilter
NT = NC_Q // 128   # 16
MC = M // 128      # 8

_CACHE = {}


def _build(nc):
    def inp(name, shape, dtype=F32):
        return nc.dram_tensor(name, shape, dtype, kind="ExternalInput")

    def outp(name, shape, dtype=F32):
        return nc.dram_tensor(name, shape, dtype, kind="ExternalOutput")

    q_d = inp("q", [2, NC_Q, D])
    qn_d = inp("qn", [2 * NC_Q, D])
    vn_d = inp("vn", [2 * NC_Q, D])
    sur_d = inp("sur", [2, NC_Q])
    wn_d = inp("wn", [2, NC_Q])
    gtd_d = inp("gtd", [2, 4])            # [g_em, 1/clip(tau), decay, 0]
    emk_d = inp("emk", [2, M, D])
    emv_d = inp("emv", [2, M, D])
    ems_d = inp("ems", [2, M])
    ema_d = inp("ema", [2, M])
    iwq_d = inp("iwq", [2, D, D])
    bq_d = inp("bq", [2, D])
    wo_d = inp("wo", [2, D, D])
    bo_d = inp("bo", [2, D])
    iota_d = inp("iota_nov", [128, NT])   # value n+1 at (p, t), n = t*128+p

    out_d = outp("out", [2, NC_Q, D])
    nk_d = outp("newK", [2, M, D])
    nv_d = outp("newV", [2, M, D])
    ns_d = outp("newS", [2, M])
    na_d = outp("newA", [2, M])
    nov_st = outp("nov_stage", [2, NC_Q])
    thr_st = outp("thr_stage", [2, NC_Q])
    cid_st = outp("cid_stage", [2, 1024])
    cid2_st = outp("cid2_stage", [2, C_CAND])
    den_st = outp("den_stage", [2, NC_Q])
    als_st = outp("als_stage", [2, M])

    sem_n = [0]

    with tile.TileContext(nc) as tc, contextlib.ExitStack() as ctx:
        cpool = ctx.enter_context(tc.tile_pool(name="consts", bufs=1))

        def roundtrip(writes, reads):
            """DRAM staging: writes then reads, ordered via explicit sem."""
            sem = nc.alloc_semaphore(f"rt{sem_n[0]}")
            sem_n[0] += 1
            with tc.tile_critical():
                nc.sync.sem_clear(sem)
                tot = 0
                for dst, src in writes:
                    nc.sync.dma_start(dst, src).then_inc(sem, 16)
                    tot += 16
                nc.sync.wait_ge(sem, tot)
                for dst, src in reads:
                    nc.sync.dma_start(dst, src).then_inc(sem, 16)
                    tot += 16
                nc.sync.wait_ge(sem, tot)

        ident = cpool.tile([128, 128], F32, name="ident")
        make_identity(nc, ident[:])
        ones_col = cpool.tile([128, 1], F32, name="onesc")
        nc.gpsimd.memset(ones_col[:], 1.0)
        ones_col_r = cpool.tile([128, 1], F32R, name="onescr")
        nc.vector.tensor_copy(ones_col_r[:], ones_col[:])
        ones_row = cpool.tile([1, 128], F32, name="onesr")
        nc.gpsimd.memset(ones_row[:], 1.0)
        iota_nov = cpool.tile([128, NT], F32, name="iotanov")
        nc.sync.dma_start(iota_nov[:], iota_d[:])

        def replicate_down(dst, src_row_ap, width):
            nc.sync.dma_start(dst[0:1, 0:width], src_row_ap)
            p = 1
            while p < 128:
                nc.sync.dma_start(dst[p:2 * p, 0:width], dst[0:p, 0:width])
                p *= 2

        for r in range(2):
            rctx = contextlib.ExitStack()
            big = rctx.enter_context(tc.tile_pool(name=f"big{r}", bufs=1))
            sn_pool = rctx.enter_context(tc.tile_pool(name=f"sn{r}", bufs=2))
            rnd_pool = rctx.enter_context(tc.tile_pool(name=f"rnd{r}", bufs=4))
            tp_pool = rctx.enter_context(tc.tile_pool(name=f"tp{r}", bufs=2))
            sm_pool = rctx.enter_context(tc.tile_pool(name=f"sm{r}", bufs=1))
            wr_pool = rctx.enter_context(tc.tile_pool(name=f"wr{r}", bufs=1))
            ps_work = rctx.enter_context(tc.tile_pool(name=f"psw{r}", bufs=2, space="PSUM"))
            ps_acc = rctx.enter_context(tc.tile_pool(name=f"psa{r}", bufs=1, space="PSUM"))
            ps_denp = rctx.enter_context(tc.tile_pool(name=f"psd{r}", bufs=1, space="PSUM"))

            def pw(shape, dtype=F32):
                return ps_work.tile(shape, dtype, tag="w", name="w")

            def col_from_scalar(val_ap, pool, n=128):
                ps = pw([n, 1])
                nc.tensor.matmul(ps[:], ones_row[:, 0:n], val_ap, start=True, stop=True)
                col = pool.tile([n, 1], F32, name="colrep", bufs=4)
                nc.scalar.copy(col[:], ps[:])
                return col
            # ============ phase A: transposed operands ============
            kT = big.tile([128, M], F32, name="kT")
            for mc in range(MC):
                t_in = tp_pool.tile([128, 128], F32, name="t_in")
                nc.sync.dma_start(t_in[:], emk_d[r, mc * 128:(mc + 1) * 128, :])
                ps = pw([128, 128])
                nc.tensor.transpose(ps[:], t_in[:], ident[:])
                nc.scalar.copy(kT[:, mc * 128:(mc + 1) * 128], ps[:])

            qT = big.tile([128, NC_Q], F32, name="qT")
            qnT = big.tile([128, NC_Q], F32, name="qnT")
            for nt in range(NT):
                t_in = tp_pool.tile([128, 128], F32, name="t_in")
                nc.sync.dma_start(t_in[:], q_d[r, nt * 128:(nt + 1) * 128, :])
                ps = pw([128, 128])
                nc.tensor.transpose(ps[:], t_in[:], ident[:])
                nc.scalar.copy(qT[:, nt * 128:(nt + 1) * 128], ps[:])
                t_in2 = tp_pool.tile([128, 128], F32, name="t_in2")
                nc.sync.dma_start(
                    t_in2[:], qn_d[r * NC_Q + nt * 128:r * NC_Q + (nt + 1) * 128, :])
                ps2 = pw([128, 128])
                nc.tensor.transpose(ps2[:], t_in2[:], ident[:])
                nc.scalar.copy(qnT[:, nt * 128:(nt + 1) * 128], ps2[:])

            ems_row = sm_pool.tile([1, M], F32, name="emsrow")
            nc.sync.dma_start(ems_row[:], ems_d[r:r + 1, :])
            act_row = sm_pool.tile([1, M], F32, name="actrow")
            nc.vector.tensor_scalar(out=act_row[:], in0=ems_row[:], scalar1=0.0,
                                    scalar2=None, op0=Alu.is_gt)
            act_rep = big.tile([128, M], F32, name="actrep")
            replicate_down(act_rep, act_row[:], M)
            kpT = big.tile([128, M], F32, name="kpT")
            nc.vector.tensor_tensor(out=kpT[:], in0=kT[:], in1=act_rep[:], op=Alu.mult)
            kpT_r = big.tile([128, M], F32R, name="kpTr")
            nc.vector.tensor_copy(kpT_r[:], kpT[:])

            iwq = tp_pool.tile([128, 128], F32, name="iwq")
            nc.sync.dma_start(iwq[:], iwq_d[r])
            iwq_r = tp_pool.tile([128, 128], F32R, name="iwqr")
            nc.vector.tensor_copy(iwq_r[:], iwq[:])
            qT_r = big.tile([128, NC_Q], F32R, name="qTr")
            nc.vector.tensor_copy(qT_r[:], qT[:])
            bq_col = sm_pool.tile([128, 1], F32, name="bqcol")
            nc.sync.dma_start(bq_col[:], bq_d[r:r + 1, :].rearrange("one d -> d one"))
            qcT_r = big.tile([128, NC_Q], F32R, name="qcTr")
            for h in range(2):
                psqc = pw([128, 1024])
                for s in range(2):
                    nc.tensor.matmul(
                        psqc[:, s * 512:(s + 1) * 512], iwq_r[:],
                        qT_r[:, h * 1024 + s * 512:h * 1024 + (s + 1) * 512],
                        start=True, stop=True)
                nc.scalar.activation(qcT_r[:, h * 1024:(h + 1) * 1024], psqc[:],
                                     Act.Identity, bias=bq_col[:, 0:1], scale=1.0)

            v_f = big.tile([128, M], F32, name="vf")
            v_r = big.tile([128, M], F32R, name="vr")
            for mc in range(MC):
                nc.sync.dma_start(v_f[:, mc * 128:(mc + 1) * 128],
                                  emv_d[r, mc * 128:(mc + 1) * 128, :])
            nc.gpsimd.tensor_copy(v_r[:], v_f[:])

            wo = tp_pool.tile([128, 128], F32, name="wo")
            nc.sync.dma_start(wo[:], wo_d[r])
            wo_r = tp_pool.tile([128, 128], F32R, name="wor")
            nc.vector.tensor_copy(wo_r[:], wo[:])
            bo_col = sm_pool.tile([128, 1], F32, name="bocol")
            nc.sync.dma_start(bo_col[:], bo_d[r:r + 1, :].rearrange("one d -> d one"))
            bo_rep = sm_pool.tile([128, 128], F32, name="borep")
            ps_bo = pw([1, 128])
            nc.tensor.transpose(ps_bo[:], bo_col[:, 0:1], ident[:])
            bo_row = sm_pool.tile([1, 128], F32, name="borow")
            nc.scalar.copy(bo_row[:], ps_bo[:])
            replicate_down(bo_rep, bo_row[:], 128)

            # ============ S^N, topk32 thr, sim, novelty ============
            thr_pt = sm_pool.tile([128, NT], F32, name="thrpt")
            msim_pt = sm_pool.tile([128, NT], F32, name="msim")
            for nt in range(NT):
                psS = pw([128, M])
                for s in range(2):
                    nc.tensor.matmul(psS[:, s * 512:(s + 1) * 512],
                                     qT[:, nt * 128:(nt + 1) * 128],
                                     kpT[:, s * 512:(s + 1) * 512],
                                     start=True, stop=True)
                s_sb = sn_pool.tile([128, M], F32, name="s_sb")
                nc.scalar.copy(s_sb[:], psS[:])
                if EXACT_THR:
                    cur = s_sb
                    for rd in range(4):
                        m8 = rnd_pool.tile([128, 8], F32, name="m8")
                        nc.vector.max(out=m8[:], in_=cur[:])
                        if rd < 3:
                            nxt = sn_pool.tile([128, M], F32, name="s_mr")
                            nc.vector.match_replace(out=nxt[:], in_to_replace=m8[:],
                                                    in_values=cur[:], imm_value=NEG_REPL)
                            cur = nxt
                else:
                    R = rnd_pool.tile([128, 256], F32, name="Rpre")
                    for c in range(32):
                        nc.vector.max(out=R[:, c * 8:(c + 1) * 8],
                                      in_=s_sb[:, c * 32:(c + 1) * 32])
                    cur = R
                    for rd in range(4):
                        m8 = rnd_pool.tile([128, 8], F32, name="m8")
                        nc.vector.max(out=m8[:], in_=cur[:])
                        if rd < 3:
                            nxt = rnd_pool.tile([128, 256], F32, name="R_mr")
                            nc.vector.match_replace(out=nxt[:], in_to_replace=m8[:],
                                                    in_values=cur[:], imm_value=NEG_REPL)
                            cur = nxt
                nc.vector.tensor_copy(thr_pt[:, nt:nt + 1], m8[:, 7:8])

                psM = pw([128, M])
                for s in range(2):
                    nc.tensor.matmul(psM[:, s * 512:(s + 1) * 512],
                                     qnT[:, nt * 128:(nt + 1) * 128],
                                     kpT[:, s * 512:(s + 1) * 512],
                                     start=True, stop=True)
                nc.vector.tensor_reduce(out=msim_pt[:, nt:nt + 1], in_=psM[:],
                                        axis=AxX, op=Alu.max)

            sur_pt = sm_pool.tile([128, NT], F32, name="surpt")
            nc.sync.dma_start(sur_pt[:], sur_d[r].rearrange("(t p) -> p t", p=128))
            wn_pt = sm_pool.tile([128, NT], F32, name="wnpt")
            nc.sync.dma_start(wn_pt[:], wn_d[r].rearrange("(t p) -> p t", p=128))
            nov_pt = sm_pool.tile([128, NT], F32, name="novpt")
            u_t = sm_pool.tile([128, NT], F32, name="u_t")
            nc.vector.tensor_scalar(out=u_t[:], in0=msim_pt[:], scalar1=0.0,
                                    scalar2=None, op0=Alu.max)
            nc.vector.tensor_scalar(out=u_t[:], in0=u_t[:], scalar1=-1.0,
                                    scalar2=-1.0, op0=Alu.mult, op1=Alu.subtract)
            w1_t = sm_pool.tile([128, NT], F32, name="w1t")
            nc.vector.tensor_scalar(out=w1_t[:], in0=wn_pt[:], scalar1=-1.0,
                                    scalar2=-1.0, op0=Alu.mult, op1=Alu.subtract)
            nc.vector.tensor_tensor(out=u_t[:], in0=u_t[:], in1=w1_t[:], op=Alu.mult)
            nc.vector.tensor_tensor(out=nov_pt[:], in0=wn_pt[:], in1=sur_pt[:],
                                    op=Alu.mult)
            nc.vector.tensor_tensor(out=nov_pt[:], in0=nov_pt[:], in1=u_t[:],
                                    op=Alu.add)

            # stage novelty + thr rows; build thr_rep
            psT = pw([NT, 128])
            nc.tensor.transpose(psT[:], nov_pt[:], ident[:])
            novT = sm_pool.tile([NT, 128], F32, name="novT")
            nc.scalar.copy(novT[:], psT[:])
            psT2 = pw([NT, 128])
            nc.tensor.transpose(psT2[:], thr_pt[:], ident[:])
            thrT = sm_pool.tile([NT, 128], F32, name="thrT")
            nc.scalar.copy(thrT[:], psT2[:])
            novrow = wr_pool.tile([1, NC_Q], F32, name="novrow")
            thr_row = sm_pool.tile([1, NC_Q], F32, name="throw")
            roundtrip(
                writes=[(nov_st[r:r + 1, :].rearrange("one (t p) -> t p", p=128), novT[:]),
                        (thr_st[r:r + 1, :].rearrange("one (t p) -> t p", p=128), thrT[:])],
                reads=[(novrow[:], nov_st[r:r + 1, :]),
                       (thr_row[:], thr_st[r:r + 1, :])])
            thr_rep = big.tile([128, NC_Q], F32, name="thrrep")
            replicate_down(thr_rep, thr_row[:], NC_Q)

            # ============ T pipeline ============
            for h in range(2):
                ps_out = ps_acc.tile([128, 1024], F32, tag="acc", name="psout")
                ps_den = ps_denp.tile([1, 1024], F32, tag="den", name="psden")
                for mc in range(MC):
                    psL = pw([128, 1024])
                    for s in range(2):
                        nc.tensor.matmul(
                            psL[:, s * 512:(s + 1) * 512],
                            kpT_r[:, mc * 128:(mc + 1) * 128],
                            qcT_r[:, h * 1024 + s * 512:h * 1024 + (s + 1) * 512],
                            start=True, stop=True)
                    psSt = pw([128, 1024])
                    for s in range(2):
                        nc.tensor.matmul(
                            psSt[:, s * 512:(s + 1) * 512],
                            kpT[:, mc * 128:(mc + 1) * 128],
                            qT[:, h * 1024 + s * 512:h * 1024 + (s + 1) * 512],
                            start=True, stop=True)
                    E_t = tp_pool.tile([128, 1024], F32, name="E_t")
                    nc.scalar.activation(E_t[:], psL[:], Act.Exp)
                    d_m = tp_pool.tile([128, 1024], F32, name="d_m")
                    nc.vector.scalar_tensor_tensor(
                        out=d_m[:], in0=psSt[:], scalar=1e-6,
                        in1=thr_rep[:, h * 1024:(h + 1) * 1024],
                        op0=Alu.add, op1=Alu.is_ge)
                    P_t = tp_pool.tile([128, 1024], F32R, name="P_t")
                    nc.gpsimd.tensor_tensor(out=P_t[:], in0=E_t[:], in1=d_m[:],
                                            op=Alu.mult)
                    for s in range(2):
                        nc.tensor.matmul(ps_out[:, s * 512:(s + 1) * 512],
                                         v_r[:, mc * 128:(mc + 1) * 128],
                                         P_t[:, s * 512:(s + 1) * 512],
                                         start=(mc == 0), stop=(mc == MC - 1))
                    for s in range(2):
                        nc.tensor.matmul(ps_den[:, s * 512:(s + 1) * 512],
                                         ones_col_r[:],
                                         P_t[:, s * 512:(s + 1) * 512],
                                         start=(mc == 0), stop=(mc == MC - 1))
                uT = tp_pool.tile([128, 1024], F32R, name="uT", bufs=1)
                nc.vector.tensor_copy(uT[:], ps_out[:])
                ps_f = ps_acc.tile([128, 1024], F32, tag="acc", name="psf")
                for s in range(2):
                    nc.tensor.matmul(ps_f[:, s * 512:(s + 1) * 512], wo_r[:],
                                     uT[:, s * 512:(s + 1) * 512],
                                     start=True, stop=True)
                fT = tp_pool.tile([128, 1024], F32, name="fT", bufs=1)
                nc.scalar.copy(fT[:], ps_f[:])
                den_row = sm_pool.tile([1, 1024], F32, name="denrow")
                nc.scalar.copy(den_row[:], ps_den[:])
                rden_pt = sm_pool.tile([128, 8], F32, name="rden")
                roundtrip(
                    writes=[(den_st[r:r + 1, h * 1024:(h + 1) * 1024], den_row[:])],
                    reads=[(rden_pt[:],
                            den_st[r, h * 1024:(h + 1) * 1024]
                            .rearrange("(t p) -> p t", p=128))])
                nc.vector.reciprocal(rden_pt[:], rden_pt[:])
                for tt in range(8):
                    nt = h * 8 + tt
                    ps_n = pw([128, 128])
                    nc.tensor.transpose(ps_n[:], fT[:, tt * 128:(tt + 1) * 128],
                                        ident[:])
                    o_sb = tp_pool.tile([128, 128], F32, name="o_sb")
                    nc.vector.scalar_tensor_tensor(
                        out=o_sb[:], in0=ps_n[:], scalar=rden_pt[:, tt:tt + 1],
                        in1=bo_rep[:], op0=Alu.mult, op1=Alu.add)
                    nc.sync.dma_start(out_d[r, nt * 128:(nt + 1) * 128, :], o_sb[:])

            # ============ write phase ============
            # t64 from novelty row (32-wide bins, top-8 each is exact enough)
            R64 = wr_pool.tile([1, 512], F32, name="R64")
            for c in range(64):
                nc.vector.max(out=R64[:, c * 8:(c + 1) * 8],
                              in_=novrow[:, c * 32:(c + 1) * 32])
            cur = R64
            for rd in range(8):
                m8w = wr_pool.tile([1, 8], F32, name="wm8", bufs=2)
                nc.vector.max(out=m8w[:], in_=cur[:])
                if rd < 7:
                    nxt = wr_pool.tile([1, 512], F32, name="R64mr", bufs=2)
                    nc.vector.match_replace(out=nxt[:], in_to_replace=m8w[:],
                                            in_values=cur[:], imm_value=NEG_REPL)
                    cur = nxt
            t64_col = col_from_scalar(m8w[:, 7:8], wr_pool)

            mgidx = sm_pool.tile([128, NT], F32, name="mgidx")
            nc.vector.tensor_scalar(out=mgidx[:], in0=nov_pt[:],
                                    scalar1=t64_col[:, 0:1], scalar2=None,
                                    op0=Alu.is_ge)
            nc.vector.tensor_tensor(out=mgidx[:], in0=mgidx[:], in1=iota_nov[:],
                                    op=Alu.mult)
            c8 = wr_pool.tile([128, 8], F32, name="c8")
            nc.vector.max(out=c8[:], in_=mgidx[:])
            cid_row = wr_pool.tile([1, 1024], F32, name="cidrow")
            roundtrip(
                writes=[(cid_st[r:r + 1, :].rearrange("one (p e) -> p e", p=128), c8[:])],
                reads=[(cid_row[:], cid_st[r:r + 1, :])])
            cur = cid_row
            cvals = wr_pool.tile([1, C_CAND], F32, name="cvals")
            for rd in range(8):
                m8b = wr_pool.tile([1, 8], F32, name="wm8b", bufs=2)
                nc.vector.max(out=m8b[:], in_=cur[:])
                nc.vector.tensor_copy(cvals[:, rd * 8:(rd + 1) * 8], m8b[:])
                if rd < 7:
                    nxt = wr_pool.tile([1, 1024], F32, name="cidmr", bufs=2)
                    nc.vector.match_replace(out=nxt[:], in_to_replace=m8b[:],
                                            in_values=cur[:], imm_value=0.0)
                    cur = nxt
            cidx_col = wr_pool.tile([C_CAND, 1], F32, name="cidxcol")
            cidx_row = wr_pool.tile([1, C_CAND], F32, name="cidxrow")
            roundtrip(
                writes=[(cid2_st[r:r + 1, :], cvals[:])],
                reads=[(cidx_col[:], cid2_st[r:r + 1, :].rearrange("one c -> c one")),
                       (cidx_row[:], cid2_st[r:r + 1, :])])
            # table index = val - 1 + r*2048, clamped into this row's range
            nc.vector.tensor_scalar(out=cidx_col[:], in0=cidx_col[:],
                                    scalar1=float(r * NC_Q - 1),
                                    scalar2=float(r * NC_Q),
                                    op0=Alu.add, op1=Alu.max)
            cidx_i = wr_pool.tile([C_CAND, 1], I32, name="cidxi")
            nc.vector.tensor_copy(cidx_i[:], cidx_col[:])

            candK = wr_pool.tile([C_CAND, D], F32, name="candK")
            nc.gpsimd.indirect_dma_start(
                out=candK[:], out_offset=None, in_=qn_d[:],
                in_offset=bass.IndirectOffsetOnAxis(ap=cidx_i[:, 0:1], axis=0))
            candV = wr_pool.tile([C_CAND, D], F32, name="candV")
            nc.gpsimd.indirect_dma_start(
                out=candV[:], out_offset=None, in_=vn_d[:],
                in_offset=bass.IndirectOffsetOnAxis(ap=cidx_i[:, 0:1], axis=0))

            # cand_scores via selection matmuls against nov_pt (no DRAM hazard)
            cidx_rep = wr_pool.tile([128, C_CAND], F32, name="cidxrep")
            replicate_down(cidx_rep, cidx_row[:], C_CAND)
            ps_cs = ps_denp.tile([C_CAND, 1], F32, tag="den", name="ps_cs")
            for nt in range(NT):
                selT = wr_pool.tile([128, C_CAND], F32, name="selT")
                nc.vector.tensor_tensor(
                    out=selT[:],
                    in0=iota_nov[:, nt:nt + 1].to_broadcast([128, C_CAND]),
                    in1=cidx_rep[:], op=Alu.is_equal)
                nc.tensor.matmul(ps_cs[:], selT[:], nov_pt[:, nt:nt + 1],
                                 start=(nt == 0), stop=(nt == NT - 1))
            cscore = wr_pool.tile([C_CAND, 1], F32, name="cscore")
            nc.scalar.copy(cscore[:], ps_cs[:])

            # normalize candK
            sq_t = wr_pool.tile([C_CAND, D], F32, name="sq_t")
            ssq = wr_pool.tile([C_CAND, 1], F32, name="ssq")
            nc.scalar.activation(sq_t[:], candK[:], Act.Square, accum_out=ssq[:])
            nrm = wr_pool.tile([C_CAND, 1], F32, name="nrm")
            nc.scalar.sqrt(nrm[:], ssq[:])
            nc.vector.tensor_scalar(out=nrm[:], in0=nrm[:], scalar1=1e-8,
                                    scalar2=None, op0=Alu.max)
            nc.vector.reciprocal(nrm[:], nrm[:])
            candKn = wr_pool.tile([C_CAND, D], F32, name="candKn")
            nc.vector.tensor_scalar(out=candKn[:], in0=candK[:], scalar1=nrm[:, 0:1],
                                    scalar2=None, op0=Alu.mult)

            # slot scores + softmax(.../tau)
            ps_kt = pw([D, C_CAND])
            nc.tensor.transpose(ps_kt[:], candKn[:], ident[0:C_CAND, 0:C_CAND])
            candKnT = wr_pool.tile([D, C_CAND], F32, name="candKnT")
            nc.scalar.copy(candKnT[:], ps_kt[:])
            emsm_row = wr_pool.tile([1, M], F32, name="emsm")
            nc.vector.tensor_scalar(out=emsm_row[:], in0=ems_row[:], scalar1=-0.5,
                                    scalar2=None, op0=Alu.mult)
            ps_sl = pw([C_CAND, M])
            for s in range(2):
                nc.tensor.matmul(ps_sl[:, s * 512:(s + 1) * 512], candKnT[:],
                                 kT[:, s * 512:(s + 1) * 512], start=True, stop=False)
                nc.tensor.matmul(ps_sl[:, s * 512:(s + 1) * 512],
                                 ones_row[0:1, 0:C_CAND],
                                 emsm_row[:, s * 512:(s + 1) * 512],
                                 start=False, stop=True)
            gtd = wr_pool.tile([1, 4], F32, name="gtd")
            nc.sync.dma_start(gtd[:], gtd_d[r:r + 1, :])
            itau_col = col_from_scalar(gtd[:, 1:2], wr_pool, n=C_CAND)
            slot_w = wr_pool.tile([C_CAND, M], F32, name="slotw")
            rowsum = wr_pool.tile([C_CAND, 1], F32, name="rowsum")
            nc.scalar.activation(slot_w[:], ps_sl[:], Act.Exp,
                                 scale=itau_col[:, 0:1], accum_out=rowsum[:])

            # wvec = g * cscore / (sum(cscore)+1e-8) / rowsum
            ps_ct = pw([1, 1])
            nc.tensor.matmul(ps_ct[:], cscore[:, 0:1], ones_col[0:C_CAND, 0:1],
                             start=True, stop=True)
            cs_tot = wr_pool.tile([1, 1], F32, name="cstot")
            nc.scalar.copy(cs_tot[:], ps_ct[:])
            nc.vector.tensor_scalar(out=cs_tot[:], in0=cs_tot[:], scalar1=1e-8,
                                    scalar2=None, op0=Alu.add)
            nc.vector.reciprocal(cs_tot[:], cs_tot[:])
            rst_col = col_from_scalar(cs_tot[:, 0:1], wr_pool, n=C_CAND)
            g_col = col_from_scalar(gtd[:, 0:1], wr_pool, n=C_CAND)
            wvec = wr_pool.tile([C_CAND, 1], F32, name="wvec")
            nc.vector.reciprocal(wvec[:], rowsum[:])
            nc.vector.tensor_tensor(out=wvec[:], in0=wvec[:], in1=cscore[:], op=Alu.mult)
            nc.vector.tensor_tensor(out=wvec[:], in0=wvec[:], in1=rst_col[:], op=Alu.mult)
            nc.vector.tensor_tensor(out=wvec[:], in0=wvec[:], in1=g_col[:], op=Alu.mult)
            alpha = wr_pool.tile([C_CAND, M], F32, name="alpha")
            nc.vector.tensor_scalar(out=alpha[:], in0=slot_w[:], scalar1=wvec[:, 0:1],
                                    scalar2=None, op0=Alu.mult)

            ps_as = ps_denp.tile([1, M], F32, tag="den", name="ps_as")
            for s in range(2):
                nc.tensor.matmul(ps_as[:, s * 512:(s + 1) * 512],
                                 ones_col[0:C_CAND, 0:1],
                                 alpha[:, s * 512:(s + 1) * 512],
                                 start=True, stop=True)
            als_row = wr_pool.tile([1, M], F32, name="alsrow")
            nc.scalar.copy(als_row[:], ps_as[:])
            als_pt = wr_pool.tile([128, MC], F32, name="alspt")
            roundtrip(writes=[(als_st[r:r + 1, :], als_row[:])],
                      reads=[(als_pt[:], als_st[r].rearrange("(c p) -> p c", p=128))])

            a_pt = wr_pool.tile([128, MC], F32, name="a_pt")
            nc.vector.tensor_scalar(out=a_pt[:], in0=als_pt[:], scalar1=1.0,
                                    scalar2=None, op0=Alu.min)
            upd_pt = wr_pool.tile([128, MC], F32, name="updpt")
            nc.vector.tensor_scalar(out=upd_pt[:], in0=als_pt[:], scalar1=1e-8,
                                    scalar2=None, op0=Alu.is_gt)
            t_pt = wr_pool.tile([128, MC], F32, name="t_pt")
            nc.vector.tensor_tensor(out=t_pt[:], in0=a_pt[:], in1=upd_pt[:], op=Alu.mult)
            omt_pt = wr_pool.tile([128, MC], F32, name="omt")
            nc.vector.tensor_scalar(out=omt_pt[:], in0=t_pt[:], scalar1=-1.0,
                                    scalar2=-1.0, op0=Alu.mult, op1=Alu.subtract)
            rden2 = wr_pool.tile([128, MC], F32, name="rden2")
            nc.vector.tensor_scalar(out=rden2[:], in0=als_pt[:], scalar1=1e-8,
                                    scalar2=None, op0=Alu.max)
            nc.vector.reciprocal(rden2[:], rden2[:])

            alpha_r = wr_pool.tile([C_CAND, M], F32R, name="alphar")
            nc.gpsimd.tensor_copy(alpha_r[:], alpha[:])
            candKn_r = wr_pool.tile([C_CAND, D], F32R, name="candKnr")
            nc.gpsimd.tensor_copy(candKn_r[:], candKn[:])
            candV_r = wr_pool.tile([C_CAND, D], F32R, name="candVr")
            nc.gpsimd.tensor_copy(candV_r[:], candV[:])

            for mt in range(MC):
                ps_bk = pw([128, D])
                nc.tensor.matmul(ps_bk[:], alpha_r[:, mt * 128:(mt + 1) * 128],
                                 candKn_r[:], start=True, stop=True)
                bl_k = tp_pool.tile([128, D], F32, name="bl_k")
                nc.scalar.copy(bl_k[:], ps_bk[:])
                ps_bv = pw([128, D])
                nc.tensor.matmul(ps_bv[:], alpha_r[:, mt * 128:(mt + 1) * 128],
                                 candV_r[:], start=True, stop=True)
                bl_v = tp_pool.tile([128, D], F32, name="bl_v")
                nc.scalar.copy(bl_v[:], ps_bv[:])
                sq2 = tp_pool.tile([128, D], F32, name="sq2")
                ssq2 = tp_pool.tile([128, 1], F32, name="ssq2")
                nc.scalar.activation(sq2[:], bl_k[:], Act.Square, accum_out=ssq2[:])
                nrm2 = tp_pool.tile([128, 1], F32, name="nrm2")
                nc.scalar.sqrt(nrm2[:], ssq2[:])
                nc.vector.tensor_scalar(out=nrm2[:], in0=nrm2[:], scalar1=1e-8,
                                        scalar2=None, op0=Alu.max)
                nc.vector.reciprocal(nrm2[:], nrm2[:])
                wk = tp_pool.tile([128, 1], F32, name="wk")
                nc.vector.tensor_tensor(out=wk[:], in0=nrm2[:], in1=t_pt[:, mt:mt + 1],
                                        op=Alu.mult)
                emk_t = tp_pool.tile([128, D], F32, name="emk_t")
                nc.sync.dma_start(emk_t[:], emk_d[r, mt * 128:(mt + 1) * 128, :])
                nk_t = tp_pool.tile([128, D], F32, name="nk_t")
                nc.vector.tensor_scalar(out=nk_t[:], in0=bl_k[:], scalar1=wk[:, 0:1],
                                        scalar2=None, op0=Alu.mult)
                nc.vector.scalar_tensor_tensor(out=nk_t[:], in0=emk_t[:],
                                               scalar=omt_pt[:, mt:mt + 1],
                                               in1=nk_t[:], op0=Alu.mult, op1=Alu.add)
                nc.sync.dma_start(nk_d[r, mt * 128:(mt + 1) * 128, :], nk_t[:])
                wv = tp_pool.tile([128, 1], F32, name="wv")
                nc.vector.tensor_tensor(out=wv[:], in0=rden2[:, mt:mt + 1],
                                        in1=t_pt[:, mt:mt + 1], op=Alu.mult)
                nv_t = tp_pool.tile([128, D], F32, name="nv_t")
                nc.vector.tensor_scalar(out=nv_t[:], in0=bl_v[:], scalar1=wv[:, 0:1],
                                        scalar2=None, op0=Alu.mult)
                nc.vector.scalar_tensor_tensor(out=nv_t[:],
                                               in0=v_f[:, mt * 128:(mt + 1) * 128],
                                               scalar=omt_pt[:, mt:mt + 1],
                                               in1=nv_t[:], op0=Alu.mult, op1=Alu.add)
                nc.sync.dma_start(nv_d[r, mt * 128:(mt + 1) * 128, :], nv_t[:])

            # new_S / new_age
            ems_pt = wr_pool.tile([128, MC], F32, name="emspt")
            nc.sync.dma_start(ems_pt[:], ems_d[r].rearrange("(c p) -> p c", p=128))
            ema_pt = wr_pool.tile([128, MC], F32, name="emapt")
            nc.sync.dma_start(ema_pt[:], ema_d[r].rearrange("(c p) -> p c", p=128))
            dec_col = col_from_scalar(gtd[:, 2:3], wr_pool)
            ns_pt = wr_pool.tile([128, MC], F32, name="ns_pt")
            nc.vector.tensor_tensor(out=ns_pt[:], in0=ems_pt[:], in1=als_pt[:],
                                    op=Alu.add)
            nc.vector.tensor_scalar(out=ns_pt[:], in0=ns_pt[:], scalar1=0.0,
                                    scalar2=S_MAX, op0=Alu.max, op1=Alu.min)
            nc.vector.tensor_scalar(out=ns_pt[:], in0=ns_pt[:],
                                    scalar1=dec_col[:, 0:1], scalar2=None,
                                    op0=Alu.mult)
            colsum = wr_pool.tile([128, 1], F32, name="colsum")
            nc.vector.tensor_reduce(out=colsum[:], in_=ns_pt[:], axis=AxX, op=Alu.add)
            ps_tot = pw([1, 1])
            nc.tensor.matmul(ps_tot[:], colsum[:, 0:1], ones_col[:, 0:1],
                             start=True, stop=True)
            tot = wr_pool.tile([1, 1], F32, name="tot")
            nc.scalar.copy(tot[:], ps_tot[:])
            nc.vector.tensor_scalar(out=tot[:], in0=tot[:], scalar1=1e-8,
                                    scalar2=None, op0=Alu.max)
            nc.vector.reciprocal(tot[:], tot[:])
            nc.vector.tensor_scalar(out=tot[:], in0=tot[:], scalar1=BUDGET,
                                    scalar2=1.0, op0=Alu.mult, op1=Alu.min)
            fac_col = col_from_scalar(tot[:, 0:1], wr_pool)
            nc.vector.tensor_scalar(out=ns_pt[:], in0=ns_pt[:],
                                    scalar1=fac_col[:, 0:1], scalar2=None,
                                    op0=Alu.mult)
            nc.sync.dma_start(ns_d[r].rearrange("(c p) -> p c", p=128), ns_pt[:])
            na_pt = wr_pool.tile([128, MC], F32, name="na_pt")
            nc.vector.tensor_scalar(out=na_pt[:], in0=als_pt[:], scalar1=-1.0,
                                    scalar2=-1.0, op0=Alu.mult, op1=Alu.subtract)
            nc.vector.tensor_tensor(out=na_pt[:], in0=ema_pt[:], in1=na_pt[:],
                                    op=Alu.mult)
            nc.sync.dma_start(na_d[r].rearrange("(c p) -> p c", p=128), na_pt[:])
            rctx.close()


def kernel(q, q_nov, v_nov, surprise, w_nov, g_em, tau, decay,
           em_K, em_V, em_S, em_age, Wq, bq, Wo, bo, C_cand):
    assert int(C_cand) == C_CAND
    arrs = dict(q=q, q_nov=q_nov, v_nov=v_nov, surprise=surprise, w_nov=w_nov,
                g_em=g_em, tau=tau, decay=decay, em_K=em_K, em_V=em_V,
                em_S=em_S, em_age=em_age)
    arrs = {k: np.ascontiguousarray(np.asarray(v), dtype=np.float32)
            for k, v in arrs.items()}
    Wq = np.asarray(Wq, np.float32)
    bq = np.asarray(bq, np.float32)
    Wo = np.asarray(Wo, np.float32)
    bo = np.asarray(bo, np.float32)
    BSB = arrs['q'].shape[0]
    n_cores = 8
    assert BSB == 16

    if 'prog' not in _CACHE:
        nc = bass.Bass("TRN2", target_bir_lowering=False, debug=False)
        _build(nc)
        _CACHE['prog'] = nc
    nc = _CACHE['prog']

    iota_nov = (np.arange(NC_Q, dtype=np.float32).reshape(NT, 128).T + 1.0).copy()
    eye = np.eye(D, dtype=np.float32)
    itau = (1.0 / np.clip(arrs['tau'], 0.01, None)).astype(np.float32)

    in_maps = []
    for c in range(n_cores):
        sl = slice(2 * c, 2 * c + 2)
        blocks = [(2 * c) % 4, (2 * c + 1) % 4]
        gtd = np.stack([
            np.array([arrs['g_em'][2 * c + i], itau[2 * c + i],
                      arrs['decay'][2 * c + i], 0.0], np.float32)
            for i in range(2)])
        in_maps.append({
            "q": arrs['q'][sl],
            "qn": np.ascontiguousarray(arrs['q_nov'][sl].reshape(2 * NC_Q, D)),
            "vn": np.ascontiguousarray(arrs['v_nov'][sl].reshape(2 * NC_Q, D)),
            "sur": arrs['surprise'][sl],
            "wn": arrs['w_nov'][sl],
            "gtd": gtd,
            "emk": arrs['em_K'][sl],
            "emv": arrs['em_V'][sl],
            "ems": arrs['em_S'][sl],
            "ema": arrs['em_age'][sl],
            "iwq": np.stack([eye + Wq[b] for b in blocks]).astype(np.float32),
            "bq": np.stack([bq[b] for b in blocks]),
            "wo": np.stack([Wo[b] for b in blocks]),
            "bo": np.stack([bo[b] for b in blocks]),
            "iota_nov": iota_nov,
        })

    res = run_bass_kernel_spmd(nc, in_maps, core_ids=list(range(n_cores)))
    _CACHE['res'] = res
    out = np.concatenate([res.results[c]["out"] for c in range(n_cores)])
    newK = np.concatenate([res.results[c]["newK"] for c in range(n_cores)])
    newV = np.concatenate([res.results[c]["newV"] for c in range(n_cores)])
    newS = np.concatenate([res.results[c]["newS"] for c in range(n_cores)])
    newA = np.concatenate([res.results[c]["newA"] for c in range(n_cores)])
    return out, newK, newV, newS, newA
